# revision 61
# baseline (speedup 1.0000x reference)
"""GSA video block kernel for 8 TRN2 NeuronCores — batch-pair sharding.

Cores pair up: group g = {2g, 2g+1} owns batch g end-to-end. Within a
group each core computes 8 heads (4 head-blocks of 2) of the gated-slot
attention over the batch's 512 tokens; a pair-local AllToAll then
redistributes head outputs to token halves for the fused out-proj + LN2
+ MLP tail (256 tokens per core, full MLP weights streamed from HBM).

The T=512 scan runs chunk-parallel (C=128) exactly as the reference:
intra-chunk causal-masked matmuls with per-slot decay, inter-chunk via
carried states K[DK,M] / V[M,DV] per head.

All transposes run on the PE array (identity matmul) — no DMA
transposes. Positive-value reciprocals use exp(-ln(x)) on the scalar
engine instead of DVE reciprocal.
"""

import os
import sys

import numpy as np
import ml_dtypes

if "/opt/trn_rl_repo" not in sys.path:
    sys.path.insert(0, "/opt/trn_rl_repo")

import concourse.bass as bass  # noqa: E402
import concourse.mybir as mybir  # noqa: E402
import concourse.tile as tile  # noqa: E402
from concourse import bacc  # noqa: E402
from concourse.bass_utils import run_bass_kernel_spmd  # noqa: E402

BF16 = mybir.dt.bfloat16
F32 = mybir.dt.float32
AF = mybir.ActivationFunctionType
ALU = mybir.AluOpType
AX = mybir.AxisListType

B, T, D = 4, 512, 1024
H, DK, DV, M = 16, 64, 64, 64
MLP = 4096
EPS = 1e-6

N_CORES = 8
C = 128                    # scan chunk length
NCH = T // C               # chunks per batch = 4
HB = 4                     # head-blocks per core (2 heads each)
TAIL = 256                 # tokens per core in the tail
DT = D // 128              # 8 d tiles
RG = [list(range(N_CORES))]

_cache = {}


def _emit(nc, tc, io):
    x_t, x_res = io["x_t"], io["x_res"]
    wq, wk, wv, wf = io["wq"], io["wk"], io["wv"], io["wf"]
    bqp, bkp, bvp, bfp = io["bqp"], io["bkp"], io["bvp"], io["bfp"]
    wo, w1, b1row, w2 = io["wo"], io["w1"], io["b1row"], io["w2"]
    ltriT, onescol, onescol1 = io["ltriT"], io["onescol"], io["onescol1"]
    cmask, ident, bd128, ones_row = (io["cmask"], io["ident"], io["bd128"],
                                     io["ones_row"])
    y_out, dump = io["y_out"], io["dump"]
    P = 128

    const = tc.alloc_tile_pool(name="const", bufs=1)
    persist = tc.alloc_tile_pool(name="persist", bufs=1)
    dram = tc.alloc_tile_pool(name="dram", bufs=1, space="DRAM")

    # ---- warmup collective (prepay ncfw handshake) -----------------------
    wa_in = dram.tile([8, 128], BF16, name="wa_in")
    wa_out = dram.tile([8, 128], BF16, name="wa_out")
    nc.gpsimd.collective_compute("AllToAll", ALU.bypass, replica_groups=RG,
                                 ins=[wa_in.opt()], outs=[wa_out.opt()])

    # ---- constants into SBUF --------------------------------------------
    def cload(ap, shape, dt, name):
        t = const.tile(shape, dt, name=name)
        nc.sync.dma_start(t[:], ap)
        return t

    ltriT_sb = cload(ltriT.ap(), [128, 128], F32, "ltriT")
    allones_sb = cload(io["allones"].ap(), [128, 128], BF16, "allones")
    onescol_sb = cload(onescol.ap(), [128, 1], F32, "onescol")
    ones1_sb = cload(onescol1.ap(), [128, 1], BF16, "ones1")
    cmask_sb = cload(cmask.ap(), [128, 128], BF16, "cmask")
    ident_sb = cload(ident.ap(), [128, 128], BF16, "ident")
    bd128_sb = cload(bd128.ap(), [128, 128], BF16, "bd128")
    ones_row_sb = cload(ones_row.ap(), [1, 128], BF16, "ones_row")
    bqp_sb = cload(bqp.ap(), [128, HB], F32, "bqp")
    bkp_sb = cload(bkp.ap(), [128, HB], F32, "bkp")
    bvp_sb = cload(bvp.ap(), [128, HB], F32, "bvp")
    bfp_sb = cload(bfp.ap(), [128, HB], F32, "bfp")
    b1row_sb = const.tile([1, MLP], BF16, name="b1row")
    eps_sb = const.tile([128, 1], F32)
    nc.vector.memset(eps_sb[:], EPS)

    # xT first on the DMA queue: stats need it before any weights
    pA = tc.alloc_tile_pool(name="pA", bufs=1)
    xT = pA.tile([128, DT, T], BF16, name="xT")
    nc.sync.dma_start(xT[:], x_t.ap().rearrange("(dt p) t -> p dt t", p=P))

    wq_sb = const.tile([128, DT, HB, 128], BF16)
    nc.sync.dma_start(wq_sb[:], wq.ap().rearrange(
        "p (dt hb j) -> p dt hb j", dt=DT, hb=HB))
    wk_sb = const.tile([128, DT, HB, 128], BF16)
    nc.sync.dma_start(wk_sb[:], wk.ap().rearrange(
        "p (dt hb j) -> p dt hb j", dt=DT, hb=HB))
    wv_sb = const.tile([128, DT, HB, 128], BF16)
    nc.sync.dma_start(wv_sb[:], wv.ap().rearrange(
        "p (dt hb j) -> p dt hb j", dt=DT, hb=HB))
    wf_sb = const.tile([128, DT, HB, 128], BF16)
    nc.sync.dma_start(wf_sb[:], wf.ap().rearrange(
        "p (dt hb j) -> p dt hb j", dt=DT, hb=HB))
    # wo_sb is loaded after the scan is emitted (it is only needed in the
    # tail) so its 2MB DMA does not delay xT/weight loads on the queue.
    wo_sb = const.tile([128, DT, D], BF16)

    # ---- persistent activation tensors ----------------------------------
    qT = persist.tile([128, HB, T], BF16, name="qT")     # [2h*64 dk, hb, t]
    kT = persist.tile([128, HB, T], BF16, name="kT")
    k_tm = persist.tile([128, HB, NCH, 128], BF16, name="k_tm")  # [t,hb,c,j]
    v_tm = persist.tile([128, HB, NCH, 128], BF16, name="v_tm")
    sp = persist.tile([128, HB, NCH, 128], F32, name="sp")       # softplus(-f)
    s_tm = persist.tile([128, HB, NCH, 128], BF16, name="s_tm")  # 1-exp(g)
    onT = persist.tile([128, HB, T], BF16, name="onT")   # normed oT

    a2a_in = dram.tile([4096, 64], BF16, name="a2a_in")
    a2a_out = dram.tile([4096, 64], BF16, name="a2a_out")

    # =====================================================================
    # Phase A: LN1 stats from xT, hT, projections, gates, tm-transposes
    # =====================================================================
    rows = tc.alloc_tile_pool(name="rows", bufs=1)

    # stats in broadcast form: all-ones lhsT puts per-token sums on every
    # partition, so the mu/var/rstd math runs full-width and no separate
    # broadcast step is needed for normalization.
    hT = pA.tile([128, DT, T], BF16, name="hT")
    with tc.tile_pool(name="stat_ps", bufs=1, space="PSUM") as stps, \
         tc.tile_pool(name="stat_sb", bufs=2) as stsb:
        ps_s = stps.tile([128, T], F32, name="ps_s")
        ps_q = stps.tile([128, T], F32, name="ps_q")
        for dt in range(DT):
            xsq = stsb.tile([128, T], BF16, name="xsq")
            nc.vector.tensor_tensor(xsq[:], xT[:, dt, :], xT[:, dt, :],
                                    ALU.mult)
            nc.tensor.matmul(ps_s[:], allones_sb[:], xT[:, dt, :],
                             start=(dt == 0), stop=(dt == DT - 1))
            nc.tensor.matmul(ps_q[:], allones_sb[:], xsq[:],
                             start=(dt == 0), stop=(dt == DT - 1))
        MU = stsb.tile([128, T], F32, name="MU")
        nc.vector.tensor_scalar_mul(MU[:], ps_s[:], 1.0 / D)
        mu2 = stsb.tile([128, T], F32, name="mu2")
        nc.vector.tensor_tensor(mu2[:], MU[:], MU[:], ALU.mult)
        var = stsb.tile([128, T], F32, name="var")
        nc.vector.tensor_scalar_mul(var[:], ps_q[:], 1.0 / D)
        nc.vector.tensor_tensor(var[:], var[:], mu2[:], ALU.subtract)
        lnv = stsb.tile([128, T], F32, name="lnv")
        nc.scalar.activation(lnv[:], var[:], AF.Ln, bias=eps_sb[:])
        RSTD = stsb.tile([128, T], F32, name="RSTD")
        nc.scalar.activation(RSTD[:], lnv[:], AF.Exp, scale=-0.5)
        for dt in range(DT):
            nc.vector.tensor_tensor(hT[:, dt, :], xT[:, dt, :], MU[:],
                                    ALU.subtract)
            nc.vector.tensor_tensor(hT[:, dt, :], hT[:, dt, :], RSTD[:],
                                    ALU.mult)

        if (d := dump("hT", [128, DT * T], BF16)) is not None:
            nc.sync.dma_start(d.ap().rearrange("p (n f) -> p n f", n=DT),
                              hT[:])

        # projections + gates + token-major transposes, per head-block
        f_tm = pA.tile([128, HB, NCH, 128], BF16, name="f_tm")
        with tc.tile_pool(name="proj_ps", bufs=3, space="PSUM") as pps, \
             tc.tile_pool(name="tr_ps", bufs=1, space="PSUM") as trp, \
             tc.tile_pool(name="pa_sb", bufs=2) as pasb:
            for hb in range(HB):
                vfh = pasb.tile([128, T], BF16, name="vfh")
                ffh = pasb.tile([128, T], BF16, name="ffh")
                for (w_sb, bias, fn, dst) in (
                        (wq_sb, bqp_sb, AF.Silu, qT[:, hb, :]),
                        (wk_sb, bkp_sb, AF.Silu, kT[:, hb, :]),
                        (wv_sb, bvp_sb, None, vfh[:]),
                        (wf_sb, bfp_sb, None, ffh[:])):
                    bank = pps.tile([128, T], F32, name="projbank")
                    for dt in range(DT):
                        nc.tensor.matmul(bank[:], w_sb[:, dt, hb, :],
                                         hT[:, dt, :],
                                         start=(dt == 0), stop=(dt == DT - 1))
                    if fn is not None:
                        nc.scalar.activation(dst, bank[:], fn,
                                             bias=bias[:, hb:hb + 1])
                    else:
                        nc.vector.tensor_scalar(dst, bank[:],
                                                bias[:, hb:hb + 1], None,
                                                ALU.add)
                # PE transposes to token-major  [t, j]
                trA = trp.tile([128, 1024], BF16, name="trA")
                trk, trv = trA[:, 0:512], trA[:, 512:1024]
                trf = trp.tile([128, 512], BF16, name="trf")
                for c in range(NCH):
                    csl = slice(c * 128, (c + 1) * 128)
                    nc.tensor.transpose(trk[:, csl], kT[:, hb, csl],
                                        ident_sb[:])
                    nc.tensor.transpose(trv[:, csl], vfh[:, csl],
                                        ident_sb[:])
                    nc.tensor.transpose(trf[:, csl], ffh[:, csl],
                                        ident_sb[:])
                for c in range(NCH):
                    csl = slice(c * 128, (c + 1) * 128)
                    nc.vector.tensor_copy(k_tm[:, hb, c, :], trk[:, csl])
                    nc.vector.tensor_copy(v_tm[:, hb, c, :], trv[:, csl])
                    nc.vector.tensor_copy(f_tm[:, hb, c, :], trf[:, csl])

            # gates, batched per activation function to avoid table reloads:
            # sp = softplus(-f) = ln(1 + exp(-f)); s = 1 - exp(-sp/8)
            enf = pA.tile([128, HB, NCH, 128], F32, name="enf")
            for hb in range(HB):
                nc.scalar.activation(enf[:, hb], f_tm[:, hb], AF.Exp,
                                     scale=-1.0)
            for hb in range(HB):
                nc.scalar.activation(sp[:, hb], enf[:, hb], AF.Ln, bias=1.0)
            e8 = pA.tile([128, HB, NCH, 128], BF16, name="e8")
            for hb in range(HB):
                nc.scalar.activation(e8[:, hb], sp[:, hb], AF.Exp,
                                     scale=-0.125)
            for hb in range(HB):
                nc.vector.tensor_scalar(s_tm[:, hb], e8[:, hb],
                                        -1.0, 1.0, ALU.mult, ALU.add)

    rows.release()
    pA.release()

    for nm, t_sb in (("qT", qT), ("kT", kT)):
        if (d := dump(nm, [128, HB * T], BF16)) is not None:
            nc.sync.dma_start(d.ap().rearrange("p (n f) -> p n f", n=HB),
                              t_sb[:])
    for nm, t_sb in (("k_tm", k_tm), ("v_tm", v_tm), ("s_tm", s_tm)):
        if (d := dump(nm, [128, HB * NCH * 128], BF16)) is not None:
            nc.sync.dma_start(
                d.ap().rearrange("p (hb c f) -> p hb c f", hb=HB, c=NCH),
                t_sb[:])
    if (d := dump("sp", [128, HB * NCH * 128])) is not None:
        nc.sync.dma_start(
            d.ap().rearrange("p (hb c f) -> p hb c f", hb=HB, c=NCH), sp[:])

    # =====================================================================
    # Phase B: chunked scan — 4 independent head-block chains per chunk
    # =====================================================================
    with tc.tile_pool(name="spsA", bufs=2, space="PSUM") as spsA, \
         tc.tile_pool(name="spsB", bufs=2, space="PSUM") as spsB, \
         tc.tile_pool(name="spsD", bufs=2, space="PSUM") as spsD, \
         tc.tile_pool(name="spsE", bufs=2, space="PSUM") as spsE, \
         tc.tile_pool(name="scan_sb", bufs=3) as ssb, \
         tc.tile_pool(name="state_sb", bufs=1) as stb:
        Kst = stb.tile([128, HB, 64], BF16, name="Kst")   # [2h*64 dk, hb, m]
        Vst = stb.tile([128, HB, 64], BF16, name="Vst")   # [2h*64 s, hb, dv]
        for c in range(NCH):
            csl = slice(c * 128, (c + 1) * 128)
            first = (c == 0)
            for hb in range(HB):
                bankA = spsA.tile([128, 512], F32, name="bankA")
                ps_b = bankA[:, 0:128]
                ps_lc = bankA[:, 128:129]
                ps_lambc = bankA[:, 132:260]
                ps_dk = bankA[:, 260:388]
                bankB = spsB.tile([128, 512], F32, name="bankB")
                ps_a = (bankB[:, 0:128], bankB[:, 128:256])
                ps_ok = bankB[:, 256:384]
                ps_dv = bankB[:, 384:512]
                bankD = spsD.tile([128, 1024], BF16, name="bankD")
                ps_pt = bankD[:, 0:128]
                ps_st = bankD[:, 128:256]
                ps_lcr = bankD[0:1, 256:384]
                bankE = spsE.tile([128, 512], F32, name="bankE")
                ps_b2 = (bankE[:, 0:128], bankE[:, 128:256])
                ps_o = (bankE[0:64, 256:384], bankE[0:64, 384:512])

                sp_t = sp[:, hb, c, :]
                # cumulative log-decay b = ltriT.T @ (-0.125 sp)  (f32)
                nc.tensor.matmul(ps_b, ltriT_sb[:], sp_t,
                                 start=True, stop=True)
                nc.tensor.matmul(ps_lc, sp_t, onescol_sb[:],
                                 start=True, stop=True)
                lam = ssb.tile([128, 128], BF16, name="lam")
                nc.scalar.activation(lam[:], ps_b, AF.Exp)
                en = ssb.tile([128, 128], BF16, name="en")
                nc.scalar.activation(en[:], ps_b, AF.Exp, scale=-1.0)
                lamCT = ssb.tile([128, 1], F32, name="lamCT")
                nc.scalar.activation(lamCT[:], ps_lc, AF.Exp)
                lamCT16 = ssb.tile([128, 1], BF16, name="lamCT16")
                nc.scalar.activation(lamCT16[:], ps_lc, AF.Exp)
                nc.tensor.transpose(ps_lcr, lamCT16[:], ident_sb[:])
                lamCr = ssb.tile([1, 128], BF16, name="lamCr")
                nc.vector.tensor_copy(lamCr[:], ps_lcr)
                nc.tensor.matmul(ps_lambc, ones_row_sb[:], lamCr[:],
                                 start=True, stop=True)

                s_til = ssb.tile([128, 128], BF16, name="s_til")
                nc.gpsimd.tensor_tensor(s_til[:], s_tm[:, hb, c, :], en[:],
                                        ALU.mult)
                s2 = ssb.tile([128, 128], BF16, name="s2")
                nc.vector.tensor_tensor(s2[:], s_til[:], ps_lambc, ALU.mult)

                am = ssb.tile([128, 256], BF16, name="am")
                for h in range(2):
                    hs = slice(h * 64, (h + 1) * 64)
                    nc.tensor.matmul(ps_a[h], kT[hs, hb, csl],
                                     qT[hs, hb, csl], start=True, stop=True)
                    nc.vector.tensor_tensor(am[:, h * 128:(h + 1) * 128],
                                            ps_a[h], cmask_sb[:], ALU.mult)
                for h in range(2):
                    hs = slice(h * 64, (h + 1) * 64)
                    oks = ps_ok[:, h * 64:(h + 1) * 64]
                    if not first:
                        nc.tensor.matmul(oks, qT[hs, hb, csl],
                                         Kst[hs, hb, :],
                                         start=True, stop=False)
                    nc.tensor.matmul(oks, am[:, h * 128:(h + 1) * 128],
                                     s_til[:, hs], start=first, stop=True)
                # slot attention weights, unnormalized: the softmax
                # denominator is a per-(head,token) scalar on o and cancels
                # exactly in the downstream per-head RMS norm.
                oksc = ssb.tile([128, 128], F32, name="oksc")
                nc.vector.tensor_tensor(oksc[:], ps_ok, lam[:], ALU.mult)
                ex = ssb.tile([128, 128], BF16, name="ex")
                nc.scalar.activation(ex[:], oksc[:], AF.Exp, scale=0.125)
                pl = ssb.tile([128, 128], BF16, name="pl")
                nc.gpsimd.tensor_tensor(pl[:], ex[:], lam[:], ALU.mult)

                # transposes: plT, s_tilT  [2h*64 s, 128 t] (both heads at
                # once: transpose swaps the (t, h*64+s) axes as needed)
                plT = ssb.tile([128, 128], BF16, name="plT")
                s_tilT = ssb.tile([128, 128], BF16, name="s_tilT")
                nc.tensor.transpose(ps_pt, pl[:], ident_sb[:])
                nc.tensor.transpose(ps_st, s_til[:], ident_sb[:])
                nc.vector.tensor_copy(plT[:], ps_pt)
                nc.vector.tensor_copy(s_tilT[:], ps_st)

                b2m = ssb.tile([128, 256], BF16, name="b2m")
                for h in range(2):
                    hs = slice(h * 64, (h + 1) * 64)
                    nc.tensor.matmul(ps_b2[h], s_tilT[hs, :], plT[hs, :],
                                     start=True, stop=True)
                    nc.vector.tensor_tensor(b2m[:, h * 128:(h + 1) * 128],
                                            ps_b2[h], cmask_sb[:], ALU.mult)
                # full-width dk/dv: only the diagonal head-blocks are used
                nc.tensor.matmul(ps_dk, k_tm[:, hb, c, :], s2[:],
                                 start=True, stop=True)
                nc.tensor.matmul(ps_dv, s2[:], v_tm[:, hb, c, :],
                                 start=True, stop=True)
                for h in range(2):
                    hs = slice(h * 64, (h + 1) * 64)
                    if not first:
                        nc.tensor.matmul(ps_o[h], Vst[hs, hb, :], plT[hs, :],
                                         start=True, stop=False)
                    nc.tensor.matmul(ps_o[h], v_tm[:, hb, c, hs],
                                     b2m[:, h * 128:(h + 1) * 128],
                                     start=first, stop=True)
                    if first:
                        nc.vector.tensor_copy(Kst[hs, hb, :],
                                              ps_dk[hs, h * 64:(h + 1) * 64])
                        nc.vector.tensor_copy(Vst[hs, hb, :],
                                              ps_dv[hs, h * 64:(h + 1) * 64])
                    else:
                        nc.vector.tensor_tensor(
                            Kst[hs, hb, :], Kst[hs, hb, :],
                            ps_lambc[hs, hs], ALU.mult)
                        nc.vector.tensor_tensor(
                            Kst[hs, hb, :], Kst[hs, hb, :],
                            ps_dk[hs, h * 64:(h + 1) * 64], ALU.add)
                        nc.vector.tensor_scalar(Vst[hs, hb, :],
                                                Vst[hs, hb, :],
                                                lamCT[hs, 0:1], None,
                                                ALU.mult)
                        nc.vector.tensor_tensor(
                            Vst[hs, hb, :], Vst[hs, hb, :],
                            ps_dv[hs, h * 64:(h + 1) * 64], ALU.add)

                nc.vector.tensor_copy(onT[0:64, hb, csl], ps_o[0])
                nc.vector.tensor_copy(onT[64:128, hb, csl], ps_o[1])

    # deferred tail-weight loads (queue is idle during the scan)
    nc.sync.dma_start(wo_sb[:], wo.ap().rearrange("p (jt n) -> p jt n",
                                                  jt=DT))
    nc.sync.dma_start(b1row_sb[:], b1row.ap())

    # =====================================================================
    # Phase C: per-head RMS over dv, then pair-local AllToAll
    # =====================================================================
    with tc.tile_pool(name="rms_ps", bufs=4, space="PSUM") as rps, \
         tc.tile_pool(name="rms_sb", bufs=4) as rsb:
        pss, lnms, rro = [], [], []
        for hb in range(HB):
            sqo = rsb.tile([128, T], BF16, name="sqo")
            nc.vector.tensor_tensor(sqo[:], onT[:, hb, :], onT[:, hb, :],
                                    ALU.mult)
            ps_ss = rps.tile([128, T], F32, name="ps_ss")
            nc.tensor.matmul(ps_ss[:], bd128_sb[:], sqo[:],
                             start=True, stop=True)
            pss.append(ps_ss)
        for hb in range(HB):
            t = rsb.tile([128, T], F32, name="lnms")
            nc.scalar.activation(t[:], pss[hb][:], AF.Ln,
                                 bias=eps_sb[:], scale=1.0 / DV)
            lnms.append(t)
        for hb in range(HB):
            t = rsb.tile([128, T], F32, name="rro")
            nc.scalar.activation(t[:], lnms[hb][:], AF.Exp, scale=-0.5)
            rro.append(t)
        for hb in range(HB):
            nc.vector.tensor_tensor(onT[:, hb, :], onT[:, hb, :],
                                    rro[hb][:], ALU.mult)

    if (d := dump("onT", [128, HB * T], BF16)) is not None:
        nc.sync.dma_start(d.ap().rearrange("p (n f) -> p n f", n=HB), onT[:])

    # head-sharded -> token-sharded redistribution. Core d's tail tokens
    # are, for every batch g, the in-batch strip [d*64, (d+1)*64): so the
    # slice this core sends to d is its 512 head-dims x that 64-token
    # strip, and every A2A slice carries useful data.
    for hb in range(HB):
        nc.sync.dma_start(
            a2a_in[:].rearrange("(dst hb p) t -> p hb dst t",
                                p=P, hb=HB)[:, hb],
            onT[:, hb, :].rearrange("p (dst t) -> p dst t", dst=8))
    nc.gpsimd.collective_compute("AllToAll", ALU.bypass, replica_groups=RG,
                                 ins=[a2a_in.opt()], outs=[a2a_out.opt()])

    # =====================================================================
    # Phase D tail: out-proj + residual + LN2 + MLP on 256 tokens
    # =====================================================================
    with tc.tile_pool(name="tail_keep", bufs=1) as tkb, \
         tc.tile_pool(name="tail_sb", bufs=2) as tsb:
        x2 = tkb.tile([128, 2, D], F32, name="x2")
        nc.sync.dma_start(x2[:],
                          x_res.ap().rearrange("(n p) d -> p n d", p=P))
        # a2a_out rows are [src=(g,i), hb, p]; tail token order is (g, t64)
        ofT = tkb.tile([128, DT, TAIL], BF16, name="ofT")
        for g in range(4):
            nc.sync.dma_start(
                ofT[:, :, g * 64:(g + 1) * 64],
                a2a_out[:].rearrange(
                    "(g i hb p) t -> p g (i hb) t", g=4, i=2, hb=HB,
                    p=P)[:, g])

        h2s = [tkb.tile([128, DT * 128], BF16, name=f"h2s{i}")
               for i in range(2)]
        with tc.tile_pool(name="op_ps", bufs=1, space="PSUM") as ops, \
             tc.tile_pool(name="h2_ps", bufs=2, space="PSUM") as h2ps:
            opb = [ops.tile([128, 512], F32, name=f"opb{i}")
                   for i in range(4)]
            for tt2 in range(2):
                t2sl = slice(tt2 * 128, (tt2 + 1) * 128)
                for jt in range(DT):
                    for nb in range(2):
                        nc.tensor.matmul(opb[tt2 * 2 + nb],
                                         ofT[:, jt, t2sl],
                                         wo_sb[:, jt,
                                               nb * 512:(nb + 1) * 512],
                                         start=(jt == 0),
                                         stop=(jt == DT - 1))
                for nb in range(2):
                    nsl = slice(nb * 512, (nb + 1) * 512)
                    nc.vector.tensor_tensor(x2[:, tt2, nsl],
                                            opb[tt2 * 2 + nb],
                                            x2[:, tt2, nsl], ALU.add)
            if (d := dump("x2", [128, 2 * D])) is not None:
                nc.sync.dma_start(d.ap().rearrange("p (n f) -> p n f", n=2),
                                  x2[:])

            # LN2 + transpose-produce h2s[tt2] = h2.T slabs.
            # Function-batched waves keep the scalar act table stable.
            ssum, ssq, mu, var, lnv2, r2 = {}, {}, {}, {}, {}, {}
            for tt2 in range(2):
                x2t = x2[:, tt2, :]
                ssum[tt2] = tsb.tile([128, 1], F32, name=f"ssum2_{tt2}")
                nc.vector.tensor_reduce(ssum[tt2][:], x2t, AX.X, ALU.add)
                sq = tsb.tile([128, D], BF16, name="sq2")
                ssq[tt2] = tsb.tile([128, 1], F32, name=f"ssq2_{tt2}")
                nc.scalar.activation(sq[:], x2t, AF.Square,
                                     accum_out=ssq[tt2][:])
            for tt2 in range(2):
                mu[tt2] = tsb.tile([128, 1], F32, name=f"mu2_{tt2}")
                nc.vector.tensor_scalar_mul(mu[tt2][:], ssum[tt2][:],
                                            1.0 / D)
                var[tt2] = tsb.tile([128, 1], F32, name=f"var2_{tt2}")
                nc.vector.tensor_tensor(var[tt2][:], mu[tt2][:],
                                        mu[tt2][:], ALU.mult)
                ex2 = tsb.tile([128, 1], F32, name="ex22")
                nc.vector.tensor_scalar_mul(ex2[:], ssq[tt2][:], 1.0 / D)
                nc.vector.tensor_tensor(var[tt2][:], ex2[:], var[tt2][:],
                                        ALU.subtract)
            for tt2 in range(2):
                lnv2[tt2] = tsb.tile([128, 1], F32, name=f"lnv2_{tt2}")
                nc.scalar.activation(lnv2[tt2][:], var[tt2][:], AF.Ln,
                                     bias=eps_sb[:])
            for tt2 in range(2):
                r2[tt2] = tsb.tile([128, 1], F32, name=f"r2_{tt2}")
                nc.scalar.activation(r2[tt2][:], lnv2[tt2][:], AF.Exp,
                                     scale=-0.5)
            for tt2 in range(2):
                negmu = tsb.tile([128, 1], F32, name="negmu")
                nc.vector.tensor_scalar_mul(negmu[:], mu[tt2][:], -1.0)
                h2t = tsb.tile([128, D], BF16, name="h2t")
                nc.vector.tensor_scalar(h2t[:], x2[:, tt2, :], negmu[:],
                                        r2[tt2][:], ALU.add, ALU.mult)
                if (dd := dump(f"h2tm{tt2}", [128, D], BF16)) is not None:
                    nc.sync.dma_start(dd.ap(), h2t[:])
                tr2 = h2ps.tile([128, 1024], BF16, name="tr2")
                for dt in range(DT):
                    nc.tensor.transpose(tr2[:, dt * 128:(dt + 1) * 128],
                                        h2t[:, dt * 128:(dt + 1) * 128],
                                        ident_sb[:])
                nc.vector.tensor_copy(h2s[tt2][:], tr2[:])

        if (d := dump("h2T", [128, 2 * DT * 128], BF16)) is not None:
            nc.sync.dma_start(d.ap().rearrange("p (i f) -> p i f", i=2)[:, 0],
                              h2s[0][:])
            nc.sync.dma_start(d.ap().rearrange("p (i f) -> p i f", i=2)[:, 1],
                              h2s[1][:])

        # MLP1 in m-quarters: y1 = h2 @ W1 + b1, gelu, PE-transpose to zT
        zs = [tkb.tile([128, MLP], BF16, name=f"zs{i}") for i in range(2)]
        with tc.tile_pool(name="mlp1_ps", bufs=1, space="PSUM") as m1ps, \
             tc.tile_pool(name="zt_ps", bufs=2, space="PSUM") as ztps, \
             tc.tile_pool(name="w1s", bufs=3) as w1s, \
             tc.tile_pool(name="z_sb", bufs=3) as zsb:
            y1b = [m1ps.tile([128, 512], F32, name=f"y1b{i}")
                   for i in range(4)]
            for mh in range(4):
                mhsl = slice(mh * 1024, (mh + 1) * 1024)
                for dt in range(DT):
                    w1d = w1s.tile([128, 1024], BF16, name="w1d")
                    nc.sync.dma_start(
                        w1d[:], w1.ap().rearrange(
                            "p (dt m) -> p dt m", dt=DT)[:, dt, mhsl])
                    for tt2 in range(2):
                        t2sl = slice(tt2 * 128, (tt2 + 1) * 128)
                        for mc in range(2):
                            bank = y1b[tt2 * 2 + mc]
                            if dt == 0:
                                nc.tensor.matmul(
                                    bank, ones_row_sb[:],
                                    b1row_sb[0:1,
                                             mh * 1024 + mc * 512:
                                             mh * 1024 + (mc + 1) * 512],
                                    start=True, stop=False)
                            nc.tensor.matmul(
                                bank, h2s[tt2][:, dt * 128:(dt + 1) * 128],
                                w1d[:, mc * 512:(mc + 1) * 512],
                                start=False, stop=(dt == DT - 1))
                for tt2 in range(2):
                    for mc in range(2):
                        zt_sb = zsb.tile([128, 512], BF16, name="zt_sb")
                        nc.scalar.activation(zt_sb[:], y1b[tt2 * 2 + mc],
                                             AF.Gelu)
                        ztr = ztps.tile([128, 512], BF16, name="ztr")
                        for q in range(4):
                            nc.tensor.transpose(
                                ztr[:, q * 128:(q + 1) * 128],
                                zt_sb[:, q * 128:(q + 1) * 128],
                                ident_sb[:])
                        msl = slice((mh * 2 + mc) * 512,
                                    (mh * 2 + mc + 1) * 512)
                        nc.vector.tensor_copy(zs[tt2][:, msl], ztr[:])

        if (d := dump("zT", [128, 2 * MLP], BF16)) is not None:
            nc.sync.dma_start(d.ap().rearrange("p (i f) -> p i f", i=2)[:, 0],
                              zs[0][:])
            nc.sync.dma_start(d.ap().rearrange("p (i f) -> p i f", i=2)[:, 1],
                              zs[1][:])

        # MLP2: y2 = z @ W2, accumulate over mt into 4 resident banks
        with tc.tile_pool(name="mlp2_ps", bufs=1, space="PSUM") as m2ps, \
             tc.tile_pool(name="w2s", bufs=4) as w2s:
            y2b = [m2ps.tile([128, 512], F32, name=f"y2b{i}")
                   for i in range(4)]
            MT = MLP // 128
            for mt in range(MT):
                w2t = w2s.tile([128, D], BF16, name="w2t")
                nc.sync.dma_start(
                    w2t[:], w2.ap().rearrange(
                        "p (mt d) -> p mt d", mt=MT)[:, mt, :])
                for tt2 in range(2):
                    for nb in range(2):
                        nc.tensor.matmul(
                            y2b[tt2 * 2 + nb],
                            zs[tt2][:, mt * 128:(mt + 1) * 128],
                            w2t[:, nb * 512:(nb + 1) * 512],
                            start=(mt == 0), stop=(mt == MT - 1))
            for tt2 in range(2):
                for nb in range(2):
                    nsl = slice(nb * 512, (nb + 1) * 512)
                    ys = tsb.tile([128, 512], F32, name="ys")
                    nc.vector.tensor_tensor(ys[:], y2b[tt2 * 2 + nb],
                                            x2[:, tt2, nsl], ALU.add)
                    nc.sync.dma_start(
                        y_out.ap().rearrange("(n p) d -> p n d", p=P)
                        [:, tt2, nsl], ys[:])

    for pool in (dram, persist, const):
        pool.release()


def _build():
    nc = bacc.Bacc("TRN2", target_bir_lowering=False, debug=False,
                   num_devices=N_CORES)

    def din(name, shape, dt=BF16):
        return nc.dram_tensor(name, shape, dt, kind="ExternalInput")

    io = dict(
        x_t=din("x_t", [D, T]),
        x_res=din("x_res", [TAIL, D], F32),
        wq=din("wq", [128, 4096]), wk=din("wk", [128, 4096]),
        wv=din("wv", [128, 4096]), wf=din("wf", [128, 4096]),
        bqp=din("bqp", [128, HB], F32), bkp=din("bkp", [128, HB], F32),
        bvp=din("bvp", [128, HB], F32), bfp=din("bfp", [128, HB], F32),
        wo=din("wo", [128, DT * D]),
        w1=din("w1", [128, DT * MLP]),
        b1row=din("b1row", [1, MLP]),
        w2=din("w2", [128, (MLP // 128) * D]),
        ltriT=din("ltriT", [128, 128], F32),
        onescol=din("onescol", [128, 1], F32),
        onescol1=din("onescol1", [128, 1]),
        allones=din("allones", [128, 128]),
        cmask=din("cmask", [128, 128]),
        ident=din("ident", [128, 128]),
        bd128=din("bd128", [128, 128]),
        ones_row=din("ones_row", [1, 128]),
        y_out=nc.dram_tensor("y_out", [TAIL, D], F32, kind="ExternalOutput"),
    )

    dbg = [s for s in os.environ.get("GSA_DEBUG", "").split(",") if s]
    dbg_outs = {}

    def dump(name, shape, dt=F32):
        if name in dbg:
            t = nc.dram_tensor("dbg_" + name, shape, dt,
                               kind="ExternalOutput")
            dbg_outs[name] = t
            return t
        return None

    io["dump"] = dump
    with tile.TileContext(nc) as tcx:
        _emit(nc, tcx, io)
    nc.compile()
    return nc, sorted(dbg_outs)


def _host_prep(inputs):
    """Fold norms/biases into weights; build per-core in_maps."""
    f32 = np.float32
    bf16 = ml_dtypes.bfloat16
    x = np.asarray(inputs["hidden_states"], f32)           # [B, T, D]
    ln1_w = np.asarray(inputs["ln1_w"], f32)
    ln1_b = np.asarray(inputs["ln1_b"], f32)
    ln2_w = np.asarray(inputs["ln2_w"], f32)
    ln2_b = np.asarray(inputs["ln2_b"], f32)
    gnorm = np.asarray(inputs["gnorm_w"], f32)
    Wq = np.asarray(inputs["Wq"], f32) * ln1_w[:, None]
    Wk = np.asarray(inputs["Wk"], f32) * ln1_w[:, None]
    Wv = np.asarray(inputs["Wv"], f32) * ln1_w[:, None]
    Wf = np.asarray(inputs["Wf"], f32) * ln1_w[:, None]
    bq = ln1_b @ np.asarray(inputs["Wq"], f32)
    bk = ln1_b @ np.asarray(inputs["Wk"], f32)
    bv = ln1_b @ np.asarray(inputs["Wv"], f32)
    bf_ = ln1_b @ np.asarray(inputs["Wf"], f32)
    Wo = np.asarray(inputs["Wo"], f32) * np.tile(gnorm, H)[:, None]
    W1 = np.asarray(inputs["W1"], f32) * ln2_w[:, None]
    b1 = np.asarray(inputs["b1"], f32) + ln2_b @ np.asarray(inputs["W1"], f32)
    W2 = np.asarray(inputs["W2"], f32)
    b2 = np.asarray(inputs["b2"], f32)

    tri = np.tril(np.ones((128, 128), f32))  # [t, tau] tau<=t
    common = dict(
        ltriT=np.ascontiguousarray((-0.125 * tri).T),        # [tau, t]
        onescol=np.full((128, 1), -0.125, f32),
        onescol1=np.ones((128, 1), bf16),
        allones=np.ones((128, 128), bf16),
        cmask=np.ascontiguousarray(tri.T.astype(bf16)),      # [tau, t]
        ident=np.eye(128, dtype=bf16),
        bd128=np.kron(np.eye(2, dtype=f32),
                      np.ones((64, 64), f32)).astype(bf16),
        ones_row=np.ones((1, 128), bf16),
        w1=np.ascontiguousarray(
            W1.reshape(DT, 128, MLP).transpose(1, 0, 2)
            .reshape(128, DT * MLP).astype(bf16)),
        b1row=np.ascontiguousarray(b1.reshape(1, MLP).astype(bf16)),
        w2=np.ascontiguousarray(
            W2.reshape(MLP // 128, 128, D).transpose(1, 0, 2)
            .reshape(128, (MLP // 128) * D).astype(bf16)),
        wo=np.ascontiguousarray(
            Wo.reshape(DT, 128, D).transpose(1, 0, 2)
            .reshape(128, DT * D).astype(bf16)),
    )
    in_maps = []
    for r in range(N_CORES):
        g, half = r // 2, r % 2
        jsl = slice(half * 512, half * 512 + 512)  # 8 heads = 512 cols
        m = dict(common)
        m["x_t"] = np.ascontiguousarray(x[g].T.astype(bf16))
        m["x_res"] = np.ascontiguousarray(
            np.concatenate([x[gg, r * 64:(r + 1) * 64] for gg in range(B)])
            + b2[None, :])
        for nm, W in (("wq", Wq), ("wk", Wk), ("wv", Wv), ("wf", Wf)):
            m[nm] = np.ascontiguousarray(
                W[:, jsl].reshape(DT, 128, HB, 128)
                .transpose(1, 0, 2, 3).reshape(128, 4096).astype(bf16))
        for nm, bvec in (("bqp", bq), ("bkp", bk), ("bvp", bv),
                         ("bfp", bf_)):
            m[nm] = np.ascontiguousarray(
                bvec[jsl].reshape(HB, 128).T.astype(f32))
        in_maps.append(m)
    return in_maps


def kernel(**inputs):
    if "nc" not in _cache:
        _cache["nc"], _cache["dbg"] = _build()
    nc = _cache["nc"]
    in_maps = _host_prep(inputs)
    res = run_bass_kernel_spmd(nc, in_maps, core_ids=list(range(N_CORES)),
                               trace=bool(os.environ.get("GSA_TRACE")))
    _cache["last_results"] = res
    out = np.zeros((B, T, D), np.float32)
    for r in range(N_CORES):
        yr = res.results[r]["y_out"]
        for g in range(B):
            out[g, r * 64:(r + 1) * 64, :] = yr[g * 64:(g + 1) * 64]
    return out


# revision 65
# speedup vs baseline: 1.1137x; 1.1137x over previous
"""GSA video block kernel for 8 TRN2 NeuronCores — batch-pair sharding.

Cores pair up: group g = {2g, 2g+1} owns batch g end-to-end. Within a
group each core computes 8 heads (4 head-blocks of 2) of the gated-slot
attention over the batch's 512 tokens; a pair-local AllToAll then
redistributes head outputs to token halves for the fused out-proj + LN2
+ MLP tail (256 tokens per core, full MLP weights streamed from HBM).

The T=512 scan runs chunk-parallel (C=128) exactly as the reference:
intra-chunk causal-masked matmuls with per-slot decay, inter-chunk via
carried states K[DK,M] / V[M,DV] per head.

All transposes run on the PE array (identity matmul) — no DMA
transposes. Positive-value reciprocals use exp(-ln(x)) on the scalar
engine instead of DVE reciprocal.
"""

import os
import sys

import numpy as np
import ml_dtypes

if "/opt/trn_rl_repo" not in sys.path:
    sys.path.insert(0, "/opt/trn_rl_repo")

import concourse.bass as bass  # noqa: E402
import concourse.mybir as mybir  # noqa: E402
import concourse.tile as tile  # noqa: E402
from concourse import bacc  # noqa: E402
from concourse.bass_utils import run_bass_kernel_spmd  # noqa: E402

BF16 = mybir.dt.bfloat16
F32 = mybir.dt.float32
AF = mybir.ActivationFunctionType
ALU = mybir.AluOpType
AX = mybir.AxisListType

B, T, D = 4, 512, 1024
H, DK, DV, M = 16, 64, 64, 64
MLP = 4096
EPS = 1e-6

N_CORES = 8
C = 128                    # scan chunk length
NCH = T // C               # chunks per batch = 4
HB = 4                     # head-blocks per core (2 heads each)
TAIL = 256                 # tokens per core in the tail
DT = D // 128              # 8 d tiles
RG = [list(range(N_CORES))]

_cache = {}


def _emit(nc, tc, io):
    x_t, x_res = io["x_t"], io["x_res"]
    wq, wk, wv, wf = io["wq"], io["wk"], io["wv"], io["wf"]
    bqp, bkp, bvp, bfp = io["bqp"], io["bkp"], io["bvp"], io["bfp"]
    wo, w1, b1row, w2 = io["wo"], io["w1"], io["b1row"], io["w2"]
    ltriT, onescol, onescol1 = io["ltriT"], io["onescol"], io["onescol1"]
    cmask, ident, bd128, ones_row = (io["cmask"], io["ident"], io["bd128"],
                                     io["ones_row"])
    y_out, dump = io["y_out"], io["dump"]
    P = 128

    const = tc.alloc_tile_pool(name="const", bufs=1)
    persist = tc.alloc_tile_pool(name="persist", bufs=1)
    dram = tc.alloc_tile_pool(name="dram", bufs=1, space="DRAM")

    # ---- warmup collective (prepay ncfw handshake) -----------------------
    wa_in = dram.tile([8, 128], BF16, name="wa_in")
    wa_out = dram.tile([8, 128], BF16, name="wa_out")
    nc.gpsimd.collective_compute("AllToAll", ALU.bypass, replica_groups=RG,
                                 ins=[wa_in.opt()], outs=[wa_out.opt()])

    # ---- constants into SBUF --------------------------------------------
    def cload(ap, shape, dt, name):
        t = const.tile(shape, dt, name=name)
        nc.sync.dma_start(t[:], ap)
        return t

    ltriT_sb = cload(ltriT.ap(), [128, 128], F32, "ltriT")
    allones_sb = cload(io["allones"].ap(), [128, 128], BF16, "allones")
    onescol_sb = cload(onescol.ap(), [128, 1], F32, "onescol")
    ones1_sb = cload(onescol1.ap(), [128, 1], BF16, "ones1")
    cmask_sb = cload(cmask.ap(), [128, 128], BF16, "cmask")
    ident_sb = cload(ident.ap(), [128, 128], BF16, "ident")
    bd128_sb = cload(bd128.ap(), [128, 128], BF16, "bd128")
    ones_row_sb = cload(ones_row.ap(), [1, 128], BF16, "ones_row")
    bqp_sb = cload(bqp.ap(), [128, HB], F32, "bqp")
    bkp_sb = cload(bkp.ap(), [128, HB], F32, "bkp")
    bvp_sb = cload(bvp.ap(), [128, HB], F32, "bvp")
    bfp_sb = cload(bfp.ap(), [128, HB], F32, "bfp")
    b1row_sb = const.tile([1, MLP], BF16, name="b1row")
    eps_sb = const.tile([128, 1], F32)
    nc.vector.memset(eps_sb[:], EPS)

    # xT first on the DMA queue: stats need it before any weights
    pA = tc.alloc_tile_pool(name="pA", bufs=1)
    xT = pA.tile([128, DT, T], BF16, name="xT")
    nc.sync.dma_start(xT[:], x_t.ap().rearrange("(dt p) t -> p dt t", p=P))

    wq_sb = const.tile([128, DT, HB, 128], BF16)
    nc.sync.dma_start(wq_sb[:], wq.ap().rearrange(
        "p (dt hb j) -> p dt hb j", dt=DT, hb=HB))
    wk_sb = const.tile([128, DT, HB, 128], BF16)
    nc.sync.dma_start(wk_sb[:], wk.ap().rearrange(
        "p (dt hb j) -> p dt hb j", dt=DT, hb=HB))
    wv_sb = const.tile([128, DT, HB, 128], BF16)
    nc.sync.dma_start(wv_sb[:], wv.ap().rearrange(
        "p (dt hb j) -> p dt hb j", dt=DT, hb=HB))
    wf_sb = const.tile([128, DT, HB, 128], BF16)
    nc.sync.dma_start(wf_sb[:], wf.ap().rearrange(
        "p (dt hb j) -> p dt hb j", dt=DT, hb=HB))
    # wo_sb is loaded after the scan is emitted (it is only needed in the
    # tail) so its 2MB DMA does not delay xT/weight loads on the queue.
    wo_sb = const.tile([128, DT, D], BF16)

    # ---- persistent activation tensors ----------------------------------
    qT = persist.tile([128, HB, T], BF16, name="qT")     # [2h*64 dk, hb, t]
    kT = persist.tile([128, HB, T], BF16, name="kT")
    k_tm = persist.tile([128, HB, NCH, 128], BF16, name="k_tm")  # [t,hb,c,j]
    v_tm = persist.tile([128, HB, NCH, 128], BF16, name="v_tm")
    sp = persist.tile([128, HB, NCH, 128], F32, name="sp")       # softplus(-f)
    s_tm = persist.tile([128, HB, NCH, 128], BF16, name="s_tm")  # 1-exp(g)
    onT = persist.tile([128, HB, T], BF16, name="onT")   # normed oT

    a2a_in = dram.tile([4096, 64], BF16, name="a2a_in")
    a2a_out = dram.tile([4096, 64], BF16, name="a2a_out")

    # =====================================================================
    # Phase A: LN1 stats from xT, hT, projections, gates, tm-transposes
    # =====================================================================
    rows = tc.alloc_tile_pool(name="rows", bufs=1)

    # stats in broadcast form: all-ones lhsT puts per-token sums on every
    # partition, so the mu/var/rstd math runs full-width and no separate
    # broadcast step is needed for normalization.
    hT = pA.tile([128, DT, T], BF16, name="hT")
    with tc.tile_pool(name="stat_ps", bufs=1, space="PSUM") as stps, \
         tc.tile_pool(name="stat_sb", bufs=2) as stsb:
        ps_s = stps.tile([128, T], F32, name="ps_s")
        ps_q = stps.tile([128, T], F32, name="ps_q")
        for dt in range(DT):
            xsq = stsb.tile([128, T], BF16, name="xsq")
            nc.gpsimd.tensor_tensor(xsq[:], xT[:, dt, :], xT[:, dt, :],
                                    ALU.mult)
            nc.tensor.matmul(ps_s[:], allones_sb[:], xT[:, dt, :],
                             start=(dt == 0), stop=(dt == DT - 1))
            nc.tensor.matmul(ps_q[:], allones_sb[:], xsq[:],
                             start=(dt == 0), stop=(dt == DT - 1))
        MU = stsb.tile([128, T], F32, name="MU")
        nc.vector.tensor_scalar_mul(MU[:], ps_s[:], 1.0 / D)
        mu2 = stsb.tile([128, T], F32, name="mu2")
        nc.vector.tensor_tensor(mu2[:], MU[:], MU[:], ALU.mult)
        var = stsb.tile([128, T], F32, name="var")
        nc.vector.tensor_scalar_mul(var[:], ps_q[:], 1.0 / D)
        nc.vector.tensor_tensor(var[:], var[:], mu2[:], ALU.subtract)
        lnv = stsb.tile([128, T], F32, name="lnv")
        nc.scalar.activation(lnv[:], var[:], AF.Ln, bias=eps_sb[:])
        RSTD = stsb.tile([128, T], F32, name="RSTD")
        nc.scalar.activation(RSTD[:], lnv[:], AF.Exp, scale=-0.5)
        for dt in range(DT):
            nc.vector.tensor_tensor(hT[:, dt, :], xT[:, dt, :], MU[:],
                                    ALU.subtract)
            nc.vector.tensor_tensor(hT[:, dt, :], hT[:, dt, :], RSTD[:],
                                    ALU.mult)

        if (d := dump("hT", [128, DT * T], BF16)) is not None:
            nc.sync.dma_start(d.ap().rearrange("p (n f) -> p n f", n=DT),
                              hT[:])

        # projections + gates + token-major transposes, per head-block
        f_tm = pA.tile([128, HB, NCH, 128], BF16, name="f_tm")
        with tc.tile_pool(name="proj_ps", bufs=3, space="PSUM") as pps, \
             tc.tile_pool(name="tr_ps", bufs=1, space="PSUM") as trp, \
             tc.tile_pool(name="pa_sb", bufs=2) as pasb:
            for hb in range(HB):
                vfh = pasb.tile([128, T], BF16, name="vfh")
                ffh = pasb.tile([128, T], BF16, name="ffh")
                for (w_sb, bias, fn, dst) in (
                        (wq_sb, bqp_sb, AF.Silu, qT[:, hb, :]),
                        (wk_sb, bkp_sb, AF.Silu, kT[:, hb, :]),
                        (wv_sb, bvp_sb, None, vfh[:]),
                        (wf_sb, bfp_sb, None, ffh[:])):
                    bank = pps.tile([128, T], F32, name="projbank")
                    for dt in range(DT):
                        nc.tensor.matmul(bank[:], w_sb[:, dt, hb, :],
                                         hT[:, dt, :],
                                         start=(dt == 0), stop=(dt == DT - 1))
                    if fn is not None:
                        nc.scalar.activation(dst, bank[:], fn,
                                             bias=bias[:, hb:hb + 1])
                    else:
                        nc.vector.tensor_scalar(dst, bank[:],
                                                bias[:, hb:hb + 1], None,
                                                ALU.add)
                # PE transposes to token-major  [t, j]
                trA = trp.tile([128, 1024], BF16, name="trA")
                trk, trv = trA[:, 0:512], trA[:, 512:1024]
                trf = trp.tile([128, 512], BF16, name="trf")
                for c in range(NCH):
                    csl = slice(c * 128, (c + 1) * 128)
                    nc.tensor.transpose(trk[:, csl], kT[:, hb, csl],
                                        ident_sb[:])
                    nc.tensor.transpose(trv[:, csl], vfh[:, csl],
                                        ident_sb[:])
                    nc.tensor.transpose(trf[:, csl], ffh[:, csl],
                                        ident_sb[:])
                for c in range(NCH):
                    csl = slice(c * 128, (c + 1) * 128)
                    nc.vector.tensor_copy(k_tm[:, hb, c, :], trk[:, csl])
                    nc.vector.tensor_copy(v_tm[:, hb, c, :], trv[:, csl])
                    nc.vector.tensor_copy(f_tm[:, hb, c, :], trf[:, csl])

            # gates, one whole-tensor instruction per function so the
            # scheduler cannot interleave and thrash the activation table:
            # sp = softplus(-f) = ln(1 + exp(-f)); s = 1 - exp(-sp/8)
            enf = pA.tile([128, HB, NCH, 128], F32, name="enf")
            nc.scalar.activation(enf[:], f_tm[:], AF.Exp, scale=-1.0)
            nc.scalar.activation(sp[:], enf[:], AF.Ln, bias=1.0)
            e8 = pA.tile([128, HB, NCH, 128], BF16, name="e8")
            nc.scalar.activation(e8[:], sp[:], AF.Exp, scale=-0.125)
            nc.vector.tensor_scalar(s_tm[:], e8[:], -1.0, 1.0,
                                    ALU.mult, ALU.add)

    rows.release()
    pA.release()

    for nm, t_sb in (("qT", qT), ("kT", kT)):
        if (d := dump(nm, [128, HB * T], BF16)) is not None:
            nc.sync.dma_start(d.ap().rearrange("p (n f) -> p n f", n=HB),
                              t_sb[:])
    for nm, t_sb in (("k_tm", k_tm), ("v_tm", v_tm), ("s_tm", s_tm)):
        if (d := dump(nm, [128, HB * NCH * 128], BF16)) is not None:
            nc.sync.dma_start(
                d.ap().rearrange("p (hb c f) -> p hb c f", hb=HB, c=NCH),
                t_sb[:])
    if (d := dump("sp", [128, HB * NCH * 128])) is not None:
        nc.sync.dma_start(
            d.ap().rearrange("p (hb c f) -> p hb c f", hb=HB, c=NCH), sp[:])

    # =====================================================================
    # Phase B: chunked scan — 4 independent head-block chains per chunk
    # =====================================================================
    with tc.tile_pool(name="spsA", bufs=2, space="PSUM") as spsA, \
         tc.tile_pool(name="spsB", bufs=2, space="PSUM") as spsB, \
         tc.tile_pool(name="spsD", bufs=2, space="PSUM") as spsD, \
         tc.tile_pool(name="spsE", bufs=2, space="PSUM") as spsE, \
         tc.tile_pool(name="scan_sb", bufs=3) as ssb, \
         tc.tile_pool(name="state_sb", bufs=1) as stb:
        Kst = stb.tile([128, HB, 64], BF16, name="Kst")   # [2h*64 dk, hb, m]
        Vst = stb.tile([128, HB, 64], BF16, name="Vst")   # [2h*64 s, hb, dv]
        for c in range(NCH):
            csl = slice(c * 128, (c + 1) * 128)
            first = (c == 0)
            for hb in range(HB):
                bankA = spsA.tile([128, 512], F32, name="bankA")
                ps_b = bankA[:, 0:128]
                ps_lc = bankA[:, 128:129]
                ps_lambc = bankA[:, 132:260]
                ps_dk = bankA[:, 260:388]
                bankB = spsB.tile([128, 512], F32, name="bankB")
                ps_a = (bankB[:, 0:128], bankB[:, 128:256])
                ps_ok = bankB[:, 256:384]
                ps_dv = bankB[:, 384:512]
                bankD = spsD.tile([128, 1024], BF16, name="bankD")
                ps_pt = bankD[:, 0:128]
                ps_st = bankD[:, 128:256]
                ps_lcr = bankD[0:1, 256:384]
                bankE = spsE.tile([128, 512], F32, name="bankE")
                ps_b2 = (bankE[:, 0:128], bankE[:, 128:256])
                ps_o = (bankE[0:64, 256:384], bankE[0:64, 384:512])

                sp_t = sp[:, hb, c, :]
                # cumulative log-decay b = ltriT.T @ (-0.125 sp)  (f32)
                nc.tensor.matmul(ps_b, ltriT_sb[:], sp_t,
                                 start=True, stop=True)
                nc.tensor.matmul(ps_lc, sp_t, onescol_sb[:],
                                 start=True, stop=True)
                lam = ssb.tile([128, 128], BF16, name="lam")
                nc.scalar.activation(lam[:], ps_b, AF.Exp)
                en = ssb.tile([128, 128], BF16, name="en")
                nc.scalar.activation(en[:], ps_b, AF.Exp, scale=-1.0)
                lamCT = ssb.tile([128, 1], F32, name="lamCT")
                nc.scalar.activation(lamCT[:], ps_lc, AF.Exp)
                lamCT16 = ssb.tile([128, 1], BF16, name="lamCT16")
                nc.scalar.activation(lamCT16[:], ps_lc, AF.Exp)
                nc.tensor.transpose(ps_lcr, lamCT16[:], ident_sb[:])
                lamCr = ssb.tile([1, 128], BF16, name="lamCr")
                nc.vector.tensor_copy(lamCr[:], ps_lcr)
                nc.tensor.matmul(ps_lambc, ones_row_sb[:], lamCr[:],
                                 start=True, stop=True)

                s_til = ssb.tile([128, 128], BF16, name="s_til")
                nc.vector.tensor_tensor(s_til[:], s_tm[:, hb, c, :], en[:],
                                        ALU.mult)
                s2 = ssb.tile([128, 128], BF16, name="s2")
                nc.vector.tensor_tensor(s2[:], s_til[:], ps_lambc, ALU.mult)

                am = ssb.tile([128, 256], BF16, name="am")
                for h in range(2):
                    hs = slice(h * 64, (h + 1) * 64)
                    nc.tensor.matmul(ps_a[h], kT[hs, hb, csl],
                                     qT[hs, hb, csl], start=True, stop=True)
                    nc.vector.tensor_tensor(am[:, h * 128:(h + 1) * 128],
                                            ps_a[h], cmask_sb[:], ALU.mult)
                for h in range(2):
                    hs = slice(h * 64, (h + 1) * 64)
                    oks = ps_ok[:, h * 64:(h + 1) * 64]
                    if not first:
                        nc.tensor.matmul(oks, qT[hs, hb, csl],
                                         Kst[hs, hb, :],
                                         start=True, stop=False)
                    nc.tensor.matmul(oks, am[:, h * 128:(h + 1) * 128],
                                     s_til[:, hs], start=first, stop=True)
                # slot attention weights, unnormalized: the softmax
                # denominator is a per-(head,token) scalar on o and cancels
                # exactly in the downstream per-head RMS norm.
                oksc = ssb.tile([128, 128], F32, name="oksc")
                nc.vector.tensor_tensor(oksc[:], ps_ok, lam[:], ALU.mult)
                ex = ssb.tile([128, 128], BF16, name="ex")
                nc.scalar.activation(ex[:], oksc[:], AF.Exp, scale=0.125)
                pl = ssb.tile([128, 128], BF16, name="pl")
                nc.vector.tensor_tensor(pl[:], ex[:], lam[:], ALU.mult)

                # transposes: plT, s_tilT  [2h*64 s, 128 t] (both heads at
                # once: transpose swaps the (t, h*64+s) axes as needed)
                plT = ssb.tile([128, 128], BF16, name="plT")
                s_tilT = ssb.tile([128, 128], BF16, name="s_tilT")
                nc.tensor.transpose(ps_pt, pl[:], ident_sb[:])
                nc.tensor.transpose(ps_st, s_til[:], ident_sb[:])
                nc.vector.tensor_copy(plT[:], ps_pt)
                nc.vector.tensor_copy(s_tilT[:], ps_st)

                b2m = ssb.tile([128, 256], BF16, name="b2m")
                for h in range(2):
                    hs = slice(h * 64, (h + 1) * 64)
                    nc.tensor.matmul(ps_b2[h], s_tilT[hs, :], plT[hs, :],
                                     start=True, stop=True)
                    nc.vector.tensor_tensor(b2m[:, h * 128:(h + 1) * 128],
                                            ps_b2[h], cmask_sb[:], ALU.mult)
                # full-width dk/dv: only the diagonal head-blocks are used
                nc.tensor.matmul(ps_dk, k_tm[:, hb, c, :], s2[:],
                                 start=True, stop=True)
                nc.tensor.matmul(ps_dv, s2[:], v_tm[:, hb, c, :],
                                 start=True, stop=True)
                for h in range(2):
                    hs = slice(h * 64, (h + 1) * 64)
                    if not first:
                        nc.tensor.matmul(ps_o[h], Vst[hs, hb, :], plT[hs, :],
                                         start=True, stop=False)
                    nc.tensor.matmul(ps_o[h], v_tm[:, hb, c, hs],
                                     b2m[:, h * 128:(h + 1) * 128],
                                     start=first, stop=True)
                    if first:
                        nc.vector.tensor_copy(Kst[hs, hb, :],
                                              ps_dk[hs, h * 64:(h + 1) * 64])
                        nc.vector.tensor_copy(Vst[hs, hb, :],
                                              ps_dv[hs, h * 64:(h + 1) * 64])
                    else:
                        nc.vector.tensor_tensor(
                            Kst[hs, hb, :], Kst[hs, hb, :],
                            ps_lambc[hs, hs], ALU.mult)
                        nc.vector.tensor_tensor(
                            Kst[hs, hb, :], Kst[hs, hb, :],
                            ps_dk[hs, h * 64:(h + 1) * 64], ALU.add)
                        nc.vector.tensor_scalar(Vst[hs, hb, :],
                                                Vst[hs, hb, :],
                                                lamCT[hs, 0:1], None,
                                                ALU.mult)
                        nc.vector.tensor_tensor(
                            Vst[hs, hb, :], Vst[hs, hb, :],
                            ps_dv[hs, h * 64:(h + 1) * 64], ALU.add)

                nc.vector.tensor_copy(onT[0:64, hb, csl], ps_o[0])
                nc.vector.tensor_copy(onT[64:128, hb, csl], ps_o[1])

    # deferred tail-weight loads (queue is idle during the scan)
    nc.sync.dma_start(wo_sb[:], wo.ap().rearrange("p (jt n) -> p jt n",
                                                  jt=DT))
    nc.sync.dma_start(b1row_sb[:], b1row.ap())

    # =====================================================================
    # Phase C: per-head RMS over dv, then pair-local AllToAll
    # =====================================================================
    with tc.tile_pool(name="rms_ps", bufs=4, space="PSUM") as rps, \
         tc.tile_pool(name="rms_sb", bufs=4) as rsb:
        pss, lnms, rro = [], [], []
        for hb in range(HB):
            sqo = rsb.tile([128, T], BF16, name="sqo")
            nc.vector.tensor_tensor(sqo[:], onT[:, hb, :], onT[:, hb, :],
                                    ALU.mult)
            ps_ss = rps.tile([128, T], F32, name="ps_ss")
            nc.tensor.matmul(ps_ss[:], bd128_sb[:], sqo[:],
                             start=True, stop=True)
            pss.append(ps_ss)
        for hb in range(HB):
            t = rsb.tile([128, T], F32, name="lnms")
            nc.scalar.activation(t[:], pss[hb][:], AF.Ln,
                                 bias=eps_sb[:], scale=1.0 / DV)
            lnms.append(t)
        for hb in range(HB):
            t = rsb.tile([128, T], F32, name="rro")
            nc.scalar.activation(t[:], lnms[hb][:], AF.Exp, scale=-0.5)
            rro.append(t)
        for hb in range(HB):
            nc.vector.tensor_tensor(onT[:, hb, :], onT[:, hb, :],
                                    rro[hb][:], ALU.mult)

    if (d := dump("onT", [128, HB * T], BF16)) is not None:
        nc.sync.dma_start(d.ap().rearrange("p (n f) -> p n f", n=HB), onT[:])

    # head-sharded -> token-sharded redistribution. Core d's tail tokens
    # are, for every batch g, the in-batch strip [d*64, (d+1)*64): so the
    # slice this core sends to d is its 512 head-dims x that 64-token
    # strip, and every A2A slice carries useful data.
    for hb in range(HB):
        nc.sync.dma_start(
            a2a_in[:].rearrange("(dst hb p) t -> p hb dst t",
                                p=P, hb=HB)[:, hb],
            onT[:, hb, :].rearrange("p (dst t) -> p dst t", dst=8))
    nc.gpsimd.collective_compute("AllToAll", ALU.bypass, replica_groups=RG,
                                 ins=[a2a_in.opt()], outs=[a2a_out.opt()])

    # =====================================================================
    # Phase D tail: out-proj + residual + LN2 + MLP on 256 tokens
    # =====================================================================
    with tc.tile_pool(name="tail_keep", bufs=1) as tkb, \
         tc.tile_pool(name="tail_sb", bufs=2) as tsb:
        x2 = tkb.tile([128, 2, D], F32, name="x2")
        nc.sync.dma_start(x2[:],
                          x_res.ap().rearrange("(n p) d -> p n d", p=P))
        # a2a_out rows are [src=(g,i), hb, p]; tail token order is (g, t64)
        ofT = tkb.tile([128, DT, TAIL], BF16, name="ofT")
        for g in range(4):
            nc.sync.dma_start(
                ofT[:, :, g * 64:(g + 1) * 64],
                a2a_out[:].rearrange(
                    "(g i hb p) t -> p g (i hb) t", g=4, i=2, hb=HB,
                    p=P)[:, g])

        h2s = [tkb.tile([128, DT * 128], BF16, name=f"h2s{i}")
               for i in range(2)]
        with tc.tile_pool(name="op_ps", bufs=1, space="PSUM") as ops, \
             tc.tile_pool(name="h2_ps", bufs=2, space="PSUM") as h2ps:
            opb = [ops.tile([128, 512], F32, name=f"opb{i}")
                   for i in range(4)]
            for tt2 in range(2):
                t2sl = slice(tt2 * 128, (tt2 + 1) * 128)
                for jt in range(DT):
                    for nb in range(2):
                        nc.tensor.matmul(opb[tt2 * 2 + nb],
                                         ofT[:, jt, t2sl],
                                         wo_sb[:, jt,
                                               nb * 512:(nb + 1) * 512],
                                         start=(jt == 0),
                                         stop=(jt == DT - 1))
                for nb in range(2):
                    nsl = slice(nb * 512, (nb + 1) * 512)
                    nc.vector.tensor_tensor(x2[:, tt2, nsl],
                                            opb[tt2 * 2 + nb],
                                            x2[:, tt2, nsl], ALU.add)
            if (d := dump("x2", [128, 2 * D])) is not None:
                nc.sync.dma_start(d.ap().rearrange("p (n f) -> p n f", n=2),
                                  x2[:])

            # LN2 + transpose-produce h2s[tt2] = h2.T slabs.
            # Function-batched waves keep the scalar act table stable.
            ssum, ssq, mu, var, lnv2, r2 = {}, {}, {}, {}, {}, {}
            for tt2 in range(2):
                x2t = x2[:, tt2, :]
                ssum[tt2] = tsb.tile([128, 1], F32, name=f"ssum2_{tt2}")
                nc.vector.tensor_reduce(ssum[tt2][:], x2t, AX.X, ALU.add)
                sq = tsb.tile([128, D], BF16, name="sq2")
                ssq[tt2] = tsb.tile([128, 1], F32, name=f"ssq2_{tt2}")
                nc.scalar.activation(sq[:], x2t, AF.Square,
                                     accum_out=ssq[tt2][:])
            for tt2 in range(2):
                mu[tt2] = tsb.tile([128, 1], F32, name=f"mu2_{tt2}")
                nc.vector.tensor_scalar_mul(mu[tt2][:], ssum[tt2][:],
                                            1.0 / D)
                var[tt2] = tsb.tile([128, 1], F32, name=f"var2_{tt2}")
                nc.vector.tensor_tensor(var[tt2][:], mu[tt2][:],
                                        mu[tt2][:], ALU.mult)
                ex2 = tsb.tile([128, 1], F32, name="ex22")
                nc.vector.tensor_scalar_mul(ex2[:], ssq[tt2][:], 1.0 / D)
                nc.vector.tensor_tensor(var[tt2][:], ex2[:], var[tt2][:],
                                        ALU.subtract)
            for tt2 in range(2):
                lnv2[tt2] = tsb.tile([128, 1], F32, name=f"lnv2_{tt2}")
                nc.scalar.activation(lnv2[tt2][:], var[tt2][:], AF.Ln,
                                     bias=eps_sb[:])
            for tt2 in range(2):
                r2[tt2] = tsb.tile([128, 1], F32, name=f"r2_{tt2}")
                nc.scalar.activation(r2[tt2][:], lnv2[tt2][:], AF.Exp,
                                     scale=-0.5)
            for tt2 in range(2):
                negmu = tsb.tile([128, 1], F32, name="negmu")
                nc.vector.tensor_scalar_mul(negmu[:], mu[tt2][:], -1.0)
                h2t = tsb.tile([128, D], BF16, name="h2t")
                nc.vector.tensor_scalar(h2t[:], x2[:, tt2, :], negmu[:],
                                        r2[tt2][:], ALU.add, ALU.mult)
                if (dd := dump(f"h2tm{tt2}", [128, D], BF16)) is not None:
                    nc.sync.dma_start(dd.ap(), h2t[:])
                tr2 = h2ps.tile([128, 1024], BF16, name="tr2")
                for dt in range(DT):
                    nc.tensor.transpose(tr2[:, dt * 128:(dt + 1) * 128],
                                        h2t[:, dt * 128:(dt + 1) * 128],
                                        ident_sb[:])
                nc.vector.tensor_copy(h2s[tt2][:], tr2[:])

        if (d := dump("h2T", [128, 2 * DT * 128], BF16)) is not None:
            nc.sync.dma_start(d.ap().rearrange("p (i f) -> p i f", i=2)[:, 0],
                              h2s[0][:])
            nc.sync.dma_start(d.ap().rearrange("p (i f) -> p i f", i=2)[:, 1],
                              h2s[1][:])

        # MLP1 in m-quarters: y1 = h2 @ W1 + b1, gelu, PE-transpose to zT
        zs = [tkb.tile([128, MLP], BF16, name=f"zs{i}") for i in range(2)]
        with tc.tile_pool(name="mlp1_ps", bufs=1, space="PSUM") as m1ps, \
             tc.tile_pool(name="zt_ps", bufs=2, space="PSUM") as ztps, \
             tc.tile_pool(name="w1s", bufs=3) as w1s, \
             tc.tile_pool(name="z_sb", bufs=3) as zsb:
            y1b = [m1ps.tile([128, 512], F32, name=f"y1b{i}")
                   for i in range(4)]
            for mh in range(4):
                mhsl = slice(mh * 1024, (mh + 1) * 1024)
                for dt in range(DT):
                    w1d = w1s.tile([128, 1024], BF16, name="w1d")
                    nc.sync.dma_start(
                        w1d[:], w1.ap().rearrange(
                            "p (dt m) -> p dt m", dt=DT)[:, dt, mhsl])
                    for tt2 in range(2):
                        t2sl = slice(tt2 * 128, (tt2 + 1) * 128)
                        for mc in range(2):
                            bank = y1b[tt2 * 2 + mc]
                            if dt == 0:
                                nc.tensor.matmul(
                                    bank, ones_row_sb[:],
                                    b1row_sb[0:1,
                                             mh * 1024 + mc * 512:
                                             mh * 1024 + (mc + 1) * 512],
                                    start=True, stop=False)
                            nc.tensor.matmul(
                                bank, h2s[tt2][:, dt * 128:(dt + 1) * 128],
                                w1d[:, mc * 512:(mc + 1) * 512],
                                start=False, stop=(dt == DT - 1))
                for tt2 in range(2):
                    for mc in range(2):
                        zt_sb = zsb.tile([128, 512], BF16, name="zt_sb")
                        nc.scalar.activation(zt_sb[:], y1b[tt2 * 2 + mc],
                                             AF.Gelu)
                        ztr = ztps.tile([128, 512], BF16, name="ztr")
                        for q in range(4):
                            nc.tensor.transpose(
                                ztr[:, q * 128:(q + 1) * 128],
                                zt_sb[:, q * 128:(q + 1) * 128],
                                ident_sb[:])
                        msl = slice((mh * 2 + mc) * 512,
                                    (mh * 2 + mc + 1) * 512)
                        nc.vector.tensor_copy(zs[tt2][:, msl], ztr[:])

        if (d := dump("zT", [128, 2 * MLP], BF16)) is not None:
            nc.sync.dma_start(d.ap().rearrange("p (i f) -> p i f", i=2)[:, 0],
                              zs[0][:])
            nc.sync.dma_start(d.ap().rearrange("p (i f) -> p i f", i=2)[:, 1],
                              zs[1][:])

        # MLP2: y2 = z @ W2, accumulate over mt into 4 resident banks
        with tc.tile_pool(name="mlp2_ps", bufs=1, space="PSUM") as m2ps, \
             tc.tile_pool(name="w2s", bufs=4) as w2s:
            y2b = [m2ps.tile([128, 512], F32, name=f"y2b{i}")
                   for i in range(4)]
            MT = MLP // 128
            for mt in range(MT):
                w2t = w2s.tile([128, D], BF16, name="w2t")
                nc.sync.dma_start(
                    w2t[:], w2.ap().rearrange(
                        "p (mt d) -> p mt d", mt=MT)[:, mt, :])
                for tt2 in range(2):
                    for nb in range(2):
                        nc.tensor.matmul(
                            y2b[tt2 * 2 + nb],
                            zs[tt2][:, mt * 128:(mt + 1) * 128],
                            w2t[:, nb * 512:(nb + 1) * 512],
                            start=(mt == 0), stop=(mt == MT - 1))
            for tt2 in range(2):
                for nb in range(2):
                    nsl = slice(nb * 512, (nb + 1) * 512)
                    ys = tsb.tile([128, 512], F32, name="ys")
                    nc.vector.tensor_tensor(ys[:], y2b[tt2 * 2 + nb],
                                            x2[:, tt2, nsl], ALU.add)
                    nc.sync.dma_start(
                        y_out.ap().rearrange("(n p) d -> p n d", p=P)
                        [:, tt2, nsl], ys[:])

    for pool in (dram, persist, const):
        pool.release()


def _build():
    nc = bacc.Bacc("TRN2", target_bir_lowering=False, debug=False,
                   num_devices=N_CORES)

    def din(name, shape, dt=BF16):
        return nc.dram_tensor(name, shape, dt, kind="ExternalInput")

    io = dict(
        x_t=din("x_t", [D, T]),
        x_res=din("x_res", [TAIL, D], F32),
        wq=din("wq", [128, 4096]), wk=din("wk", [128, 4096]),
        wv=din("wv", [128, 4096]), wf=din("wf", [128, 4096]),
        bqp=din("bqp", [128, HB], F32), bkp=din("bkp", [128, HB], F32),
        bvp=din("bvp", [128, HB], F32), bfp=din("bfp", [128, HB], F32),
        wo=din("wo", [128, DT * D]),
        w1=din("w1", [128, DT * MLP]),
        b1row=din("b1row", [1, MLP]),
        w2=din("w2", [128, (MLP // 128) * D]),
        ltriT=din("ltriT", [128, 128], F32),
        onescol=din("onescol", [128, 1], F32),
        onescol1=din("onescol1", [128, 1]),
        allones=din("allones", [128, 128]),
        cmask=din("cmask", [128, 128]),
        ident=din("ident", [128, 128]),
        bd128=din("bd128", [128, 128]),
        ones_row=din("ones_row", [1, 128]),
        y_out=nc.dram_tensor("y_out", [TAIL, D], F32, kind="ExternalOutput"),
    )

    dbg = [s for s in os.environ.get("GSA_DEBUG", "").split(",") if s]
    dbg_outs = {}

    def dump(name, shape, dt=F32):
        if name in dbg:
            t = nc.dram_tensor("dbg_" + name, shape, dt,
                               kind="ExternalOutput")
            dbg_outs[name] = t
            return t
        return None

    io["dump"] = dump
    with tile.TileContext(nc) as tcx:
        _emit(nc, tcx, io)
    nc.compile()
    return nc, sorted(dbg_outs)


def _host_prep(inputs):
    """Fold norms/biases into weights; build per-core in_maps."""
    f32 = np.float32
    bf16 = ml_dtypes.bfloat16
    x = np.asarray(inputs["hidden_states"], f32)           # [B, T, D]
    ln1_w = np.asarray(inputs["ln1_w"], f32)
    ln1_b = np.asarray(inputs["ln1_b"], f32)
    ln2_w = np.asarray(inputs["ln2_w"], f32)
    ln2_b = np.asarray(inputs["ln2_b"], f32)
    gnorm = np.asarray(inputs["gnorm_w"], f32)
    Wq = np.asarray(inputs["Wq"], f32) * ln1_w[:, None]
    Wk = np.asarray(inputs["Wk"], f32) * ln1_w[:, None]
    Wv = np.asarray(inputs["Wv"], f32) * ln1_w[:, None]
    Wf = np.asarray(inputs["Wf"], f32) * ln1_w[:, None]
    bq = ln1_b @ np.asarray(inputs["Wq"], f32)
    bk = ln1_b @ np.asarray(inputs["Wk"], f32)
    bv = ln1_b @ np.asarray(inputs["Wv"], f32)
    bf_ = ln1_b @ np.asarray(inputs["Wf"], f32)
    Wo = np.asarray(inputs["Wo"], f32) * np.tile(gnorm, H)[:, None]
    W1 = np.asarray(inputs["W1"], f32) * ln2_w[:, None]
    b1 = np.asarray(inputs["b1"], f32) + ln2_b @ np.asarray(inputs["W1"], f32)
    W2 = np.asarray(inputs["W2"], f32)
    b2 = np.asarray(inputs["b2"], f32)

    tri = np.tril(np.ones((128, 128), f32))  # [t, tau] tau<=t
    common = dict(
        ltriT=np.ascontiguousarray((-0.125 * tri).T),        # [tau, t]
        onescol=np.full((128, 1), -0.125, f32),
        onescol1=np.ones((128, 1), bf16),
        allones=np.ones((128, 128), bf16),
        cmask=np.ascontiguousarray(tri.T.astype(bf16)),      # [tau, t]
        ident=np.eye(128, dtype=bf16),
        bd128=np.kron(np.eye(2, dtype=f32),
                      np.ones((64, 64), f32)).astype(bf16),
        ones_row=np.ones((1, 128), bf16),
        w1=np.ascontiguousarray(
            W1.reshape(DT, 128, MLP).transpose(1, 0, 2)
            .reshape(128, DT * MLP).astype(bf16)),
        b1row=np.ascontiguousarray(b1.reshape(1, MLP).astype(bf16)),
        w2=np.ascontiguousarray(
            W2.reshape(MLP // 128, 128, D).transpose(1, 0, 2)
            .reshape(128, (MLP // 128) * D).astype(bf16)),
        wo=np.ascontiguousarray(
            Wo.reshape(DT, 128, D).transpose(1, 0, 2)
            .reshape(128, DT * D).astype(bf16)),
    )
    in_maps = []
    for r in range(N_CORES):
        g, half = r // 2, r % 2
        jsl = slice(half * 512, half * 512 + 512)  # 8 heads = 512 cols
        m = dict(common)
        m["x_t"] = np.ascontiguousarray(x[g].T.astype(bf16))
        m["x_res"] = np.ascontiguousarray(
            np.concatenate([x[gg, r * 64:(r + 1) * 64] for gg in range(B)])
            + b2[None, :])
        for nm, W in (("wq", Wq), ("wk", Wk), ("wv", Wv), ("wf", Wf)):
            m[nm] = np.ascontiguousarray(
                W[:, jsl].reshape(DT, 128, HB, 128)
                .transpose(1, 0, 2, 3).reshape(128, 4096).astype(bf16))
        for nm, bvec in (("bqp", bq), ("bkp", bk), ("bvp", bv),
                         ("bfp", bf_)):
            m[nm] = np.ascontiguousarray(
                bvec[jsl].reshape(HB, 128).T.astype(f32))
        in_maps.append(m)
    return in_maps


def kernel(**inputs):
    if "nc" not in _cache:
        _cache["nc"], _cache["dbg"] = _build()
    nc = _cache["nc"]
    in_maps = _host_prep(inputs)
    res = run_bass_kernel_spmd(nc, in_maps, core_ids=list(range(N_CORES)),
                               trace=bool(os.environ.get("GSA_TRACE")))
    _cache["last_results"] = res
    out = np.zeros((B, T, D), np.float32)
    for r in range(N_CORES):
        yr = res.results[r]["y_out"]
        for g in range(B):
            out[g, r * 64:(r + 1) * 64, :] = yr[g * 64:(g + 1) * 64]
    return out
